# revision 30
# baseline (speedup 1.0000x reference)
"""Trainium2 Bass kernel for nn_C2FTransformerV13 (dense transformer).

Sharding: 2-way data parallel over batch x 4-way tensor parallel
(5 attention heads / core, 640 FFN hidden / core), groups [[0-3],[4-7]],
AllReduce after attention out-proj and after FFN w2.

Layout: activations kept feature-major ("x_T" = [D, S]) so every matmul
contracts along the partition axis with zero transposes. The per-token
1/rms scale commutes through the linear q/k/v maps, so projections start
from raw x and the scale is fused into the PSUM->SBUF copies (q/k) or the
per-k h tiles (v).

Pipelining: the two per-layer TP all-reduces run in fp16 and are split
into k-halves (k0-3 / k4-9).  Production is interleaved (wo m0-3
accumulates while attention pairs complete; the first AR half fires as
soon as those 4 partial tiles are done) and consumption is k-split (w1
accumulates k0-3 while the second AR half is in flight; next layer's
rms/q-proj accumulate k0-3 of x while the FFN AR's second half flies).
PSUM->SBUF partial copies run on the scalar engine; reciprocals use the
fast approx DVE op; gpsimd only fires collective triggers.
"""

import sys

for _p in ("/opt/trn_rl_repo", "/root/.axon_site/_ro/trn_rl_repo"):
    if _p not in sys.path:
        sys.path.insert(0, _p)

import math
import numpy as np
import ml_dtypes

BF16 = ml_dtypes.bfloat16

# ---- model dims (hardcoded from the problem spec) ----
B, S = 2, 512
D, H, L = 1280, 20, 16
DK = D // H                      # 64
LAT_C = 112
VOCAB, NPRED = 1024, 10
D_INNER = 2 * D                  # 2560
NUM_BUCKETS, MAX_DIST = 32, 128

N_CORES = 8
TP = 4
GROUPS = [[0, 1, 2, 3], [4, 5, 6, 7]]
HL = H // TP                     # 5 local heads
QS = HL * DK                     # 320 local qkv cols
FS = D_INNER // TP               # 640 local ffn hidden
CS = VOCAB * NPRED // TP         # 2560 local cls rows
P = 128
NK = D // P                      # 10 k-tiles over D
NKA = 4                          # first AR half: k0-3
NKB = NK - NKA                   # second AR half: k4-9
NMQ = 3                          # q (or k) M-tiles (320 -> padded 384)
NMC = CS // P                    # 20 cls M-tiles
EXP_BIAS = -12.0                 # constant shift inside exp; cancels in softmax
EPS = 1e-5


# ------------------------------------------------------------------
# host-side prep
# ------------------------------------------------------------------

def _relative_buckets(s):
    ctx = np.arange(s)[:, None]
    mem = np.arange(s)[None, :]
    n = -(mem - ctx)
    nb = NUM_BUCKETS // 2
    ret = (n < 0).astype(np.int64) * nb
    n = np.abs(n)
    max_exact = nb // 2
    is_small = n < max_exact
    safe = np.maximum(n, 1).astype(np.float32)
    val_large = max_exact + (
        np.log(safe / max_exact) / math.log(MAX_DIST / max_exact) * (nb - max_exact)
    ).astype(np.int64)
    val_large = np.minimum(val_large, nb - 1)
    return ret + np.where(is_small, n, val_large)


def _img_kxm(w, nk, nm):
    """[K=nk*128, M=nm*128] -> [128, nm*nk, 128]; tile (m,k) at index m*nk+k."""
    assert w.shape == (nk * P, nm * P)
    return np.ascontiguousarray(
        w.reshape(nk, P, nm, P).transpose(1, 2, 0, 3).reshape(P, nm * nk, P)
    )


def _pad_cols(w, to):
    out = np.zeros((w.shape[0], to), w.dtype)
    out[:, : w.shape[1]] = w
    return out


def _pad_rows(w, to):
    out = np.zeros((to, w.shape[1]), w.dtype)
    out[: w.shape[0]] = w
    return out


def make_in_maps(inputs):
    """inputs: full f32 arrays keyed as in setup_inputs(). Returns in_maps[8]."""
    latents = np.asarray(inputs["latents"], np.float32)
    emb_w = np.asarray(inputs["emb_w"], np.float32)
    emb_b = np.asarray(inputs["emb_b"], np.float32)
    norm_w = np.asarray(inputs["norm_w"], np.float32)
    wq = np.asarray(inputs["wq"], np.float32)
    wk = np.asarray(inputs["wk"], np.float32)
    wv = np.asarray(inputs["wv"], np.float32)
    wo = np.asarray(inputs["wo"], np.float32)
    w1 = np.asarray(inputs["w1"], np.float32)
    w2 = np.asarray(inputs["w2"], np.float32)
    rel_bias = np.asarray(inputs["rel_bias"], np.float32)
    norm_out_w = np.asarray(inputs["norm_out_w"], np.float32)
    cls_w = np.asarray(inputs["cls_w"], np.float32)
    cls_b = np.asarray(inputs["cls_b"], np.float32)

    pb = rel_bias[_relative_buckets(S)]        # [sq, sk, H]

    in_maps = []
    rank_cache = []
    for r in range(TP):
        qc = slice(r * QS, (r + 1) * QS)
        fc = slice(r * FS, (r + 1) * FS)
        gc = slice(D_INNER + r * FS, D_INNER + (r + 1) * FS)
        cc = slice(r * CS, (r + 1) * CS)

        wqk_l, wv_l, wo_l, w1_l, w2_l = [], [], [], [], []
        for l in range(L):
            nw = norm_w[l][:, None]
            wq_eff = (nw * wq[l] / math.sqrt(DK))[:, qc]      # [1280, 320]
            wk_eff = (nw * wk[l])[:, qc]
            wv_eff = (nw * wv[l])[:, qc]
            q_img = _img_kxm(_pad_cols(wq_eff, NMQ * P), NK, NMQ)
            k_img = _img_kxm(_pad_cols(wk_eff, NMQ * P), NK, NMQ)
            wqk_l.append(np.concatenate([q_img, k_img], axis=1))  # [128,60,128]
            wv_l.append(
                np.ascontiguousarray(wv_eff.reshape(NK, P, QS).transpose(1, 0, 2))
            )  # [128, 10, 320]
            wo_l.append(_img_kxm(_pad_rows(wo[l][qc, :], 3 * P), 3, NK))
            w1_sh = np.concatenate([w1[l][:, fc], w1[l][:, gc]], axis=1)
            w1_l.append(_img_kxm(w1_sh, NK, NK))                # [128,100,128]
            w2_l.append(_img_kxm(w2[l][fc.start : fc.stop, :], 5, NK))

        cls_eff = (cls_w[cc, :] * norm_out_w[None, :]).T        # [1280, 2560]
        cls_img = _img_kxm(cls_eff, NK, NMC)                    # [128, 200, 128]
        clsb_img = np.ascontiguousarray(cls_b[cc].reshape(NMC, P).T)

        posb = np.ascontiguousarray(
            pb[:, :, r * HL : (r + 1) * HL].transpose(2, 1, 0)  # [5, sk, sq]
        ).reshape(HL, 4, P, S)

        rank_cache.append(
            dict(
                wqk=np.stack(wqk_l).astype(np.float32),
                wv=np.stack(wv_l).astype(np.float32),
                wo=np.stack(wo_l).astype(np.float32),
                w1=np.stack(w1_l).astype(np.float32),
                w2=np.stack(w2_l).astype(np.float32),
                cls=cls_img.astype(np.float32),
                clsb=clsb_img.astype(np.float32),
                posb=posb.astype(BF16),
            )
        )

    embw_img = np.ascontiguousarray(emb_w.T).astype(np.float32)   # [112, 1280]
    embb_img = np.ascontiguousarray(emb_b.reshape(NK, P).T).astype(np.float32)

    for c in range(N_CORES):
        g, r = divmod(c, TP)
        m = dict(rank_cache[r])
        m["embw"] = embw_img
        m["embb"] = embb_img
        m["lat"] = latents[g].astype(np.float32)                  # [112, 512]
        in_maps.append(m)
    return in_maps


# ------------------------------------------------------------------
# device kernel
# ------------------------------------------------------------------

_BUILD_CACHE = {}


def build_nc():
    if "nc" in _BUILD_CACHE:
        return _BUILD_CACHE["nc"]

    import concourse.mybir as mybir
    import concourse.tile as tile
    from concourse import bacc

    dt = mybir.dt
    AF = mybir.ActivationFunctionType
    f32, f32r, bf16 = dt.float32, dt.float32r, dt.bfloat16
    f16 = dt.float16

    nc = bacc.Bacc("TRN2", target_bir_lowering=False, debug=False,
                   num_devices=N_CORES)

    wqk_d = nc.dram_tensor("wqk", [L, P, 60, P], f32r, kind="ExternalInput")
    wv_d = nc.dram_tensor("wv", [L, P, NK, QS], f32r, kind="ExternalInput")
    wo_d = nc.dram_tensor("wo", [L, P, 30, P], f32r, kind="ExternalInput")
    w1_d = nc.dram_tensor("w1", [L, P, 100, P], f32r, kind="ExternalInput")
    w2_d = nc.dram_tensor("w2", [L, P, 50, P], f32r, kind="ExternalInput")
    cls_d = nc.dram_tensor("cls", [P, 200, P], f32r, kind="ExternalInput")
    clsb_d = nc.dram_tensor("clsb", [P, NMC], f32, kind="ExternalInput")
    embw_d = nc.dram_tensor("embw", [LAT_C, D], f32r, kind="ExternalInput")
    embb_d = nc.dram_tensor("embb", [P, NK], f32, kind="ExternalInput")
    lat_d = nc.dram_tensor("lat", [LAT_C, S], f32r, kind="ExternalInput")
    posb_d = nc.dram_tensor("posb", [HL, 4, P, S], bf16, kind="ExternalInput")

    out_d = nc.dram_tensor("out", [NMC, P, S], f32, kind="ExternalOutput")

    with tile.TileContext(nc) as tc:
        import contextlib

        ctx = contextlib.ExitStack()
        with ctx:
            sp = lambda name, bufs: ctx.enter_context(
                tc.tile_pool(name=name, bufs=bufs)
            )
            static = sp("static", 1)
            x_sb = static.tile([P, NK, S], f32r, name="x_sb")
            embb_sb = static.tile([P, NK], f32, name="embb_sb")
            clsb_sb = static.tile([P, NMC], f32, name="clsb_sb")
            ones_c = static.tile([P, 1], f32r, name="ones_c")
            ones_r = static.tile([1, P], f32r, name="ones_r")
            eps_t = static.tile([1, 1], f32, name="eps_t")
            ebias_t = static.tile([P, 1], f32, name="ebias_t")
            warm_sb = static.tile([1, 64], f32, name="warm_sb")
            warm_mid = static.tile([1, 64], f32, name="warm_mid")

            big_p = sp("big_p", 2)      # attn-partial / ffn-partial (fp16)
            qk_p = sp("qk_p", 2)        # q and k live simultaneously
            v_p = sp("v_p", 1)
            on_p = sp("on_p", 1)
            ff_p = sp("ff_p", 1)
            wqk_p = sp("wqk_p", 4)      # [128,10,128] f32r per M-tile
            wv_p = sp("wv_p", 1)        # whole-layer (each k-slice read 4x)
            wo_p = sp("wo_p", 4)        # [128,3,128]
            w1_p = sp("w1_p", 4)        # [128,10,128]
            w2_p = sp("w2_p", 4)        # [128,5,128]
            clsw_p = sp("clsw_p", 2)    # [128,10,128]
            sq_p = sp("sq_p", 2)
            e_p = sp("e_p", 2)
            pb_p = sp("pb_p", 2)
            sc1_p = sp("sc1_p", 2)      # [1,512] scratch
            bcs_p = sp("bcs_p", 2)      # bc (1/rms bcast) SBUF copy
            rb_p = sp("rb_p", 2)        # [64,512] recip-bcast sbuf
            onorm_p = sp("onorm_p", 2)  # [64,512] odd-head scratch
            ga_p = sp("ga_p", 2)        # gelu(g)
            ar_p = sp("ar_p", 2)        # AR-return [128,512] fp16
            outw_p = sp("outw_p", 2)    # cls output staging

            ps = ctx.enter_context(tc.tile_pool(name="ps", bufs=4, space="PSUM"))
            dram = ctx.enter_context(tc.tile_pool(name="dram", bufs=2, space="DRAM"))

            nc.vector.memset(ones_c[:].bitcast(f32), 1.0)
            nc.vector.memset(ones_r[:].bitcast(f32), 1.0)
            nc.vector.memset(eps_t[:], EPS)
            nc.vector.memset(ebias_t[:], EXP_BIAS)
            nc.vector.memset(warm_sb[:], 1.0)

            def warm_chain(n, nm):
                # tiny fp32 matmuls self-chained through gpsimd-queue DMA
                # round-trips (~1.5us/hop): keeps the PE HAM clock at 8/8
                # through an AllReduce window without blocking the DVE/
                # scalar/sync queues the real pipeline uses
                for i in range(n):
                    wp = ps.tile([1, 64], f32, tag="sc", bufs=2,
                                 name=f"warm_{nm}_{i}")
                    nc.tensor.matmul(wp[:], ones_r[0:1, 0:1].bitcast(f32),
                                     warm_sb[:], start=True, stop=True)
                    nc.scalar.copy(warm_mid[:], wp[:])
                    nc.gpsimd.dma_start(warm_sb[:], warm_mid[:])

            # embedding weight/latents borrow the fp16 partials pool buffers
            # (they are only live before layer 0's partials exist)
            emb_sb = big_p.tile([LAT_C, D], f32r, tag="big", name="emb_sb")
            lat_sb = big_p.tile([LAT_C, S], f32r, tag="big", name="lat_sb")
            nc.sync.dma_start(emb_sb[:], embw_d.ap())
            nc.sync.dma_start(lat_sb[:], lat_d.ap())
            nc.sync.dma_start(embb_sb[:], embb_d.ap())
            nc.sync.dma_start(clsb_sb[:], clsb_d.ap())

            # ---------------- embedding ----------------
            for m in range(NK):
                acc = ps.tile([P, S], f32, tag="acc")
                nc.tensor.matmul(acc[:], emb_sb[:, m * P : (m + 1) * P],
                                 lat_sb[:], start=True, stop=True)
                nc.vector.tensor_scalar_add(x_sb[:, m, :], acc[:],
                                            embb_sb[:, m : m + 1])

            def res_adds(prev, lo, hi, nm):
                """x[:, j, :] += AR output tile j for j in [lo, hi)."""
                if prev is None:
                    return
                src_a, src_b = prev
                for j in range(lo, hi):
                    src = src_a if j < NKA else src_b
                    art = ar_p.tile([P, S], f16, tag="ar",
                                    name=f"art_{nm}_{j}")
                    nc.sync.dma_start(
                        art[:], src[:][j if j < NKA else j - NKA])
                    nc.vector.tensor_add(x_sb[:, j, :], x_sb[:, j, :], art[:])

            def fire_ar(part, lo, hi, nm):
                n = hi - lo
                ci = dram.tile([n, P, S], f16, tag=f"ci{lo}", name=f"ci_{nm}")
                co = dram.tile([n, P, S], f16, tag=f"co{lo}", name=f"co_{nm}")
                nc.sync.dma_start(ci[:].rearrange("t p n -> p t n"),
                                  part[:, lo:hi, :])
                nc.gpsimd.collective_compute(
                    "AllReduce", mybir.AluOpType.add, replica_groups=GROUPS,
                    ins=[ci[:]], outs=[co[:]],
                )
                return co

            # ---------------- layers ----------------
            prev = None            # (cc_out_a k0-3, cc_out_b k4-9) of AR2[l-1]
            for l in range(L):
                wv = wv_p.tile([P, NK, QS], f32r, tag="wv", name=f"wv_{l}")
                nc.sync.dma_start(wv[:], wv_d.ap()[l])

                # === epilogue of AR2[l-1] (a-half) + k-split rms/q start ===
                if prev is not None:
                    warm_chain(10, f"a2_{l}")
                res_adds(prev, 0, NKA, f"x2a_{l}")
                ss = ps.tile([1, S], f32, tag="acc", name=f"ss_{l}")
                for j in range(NKA):
                    sq = sq_p.tile([P, S], f32r, tag="sq")
                    nc.vector.tensor_mul(sq[:], x_sb[:, j, :], x_sb[:, j, :])
                    nc.tensor.matmul(ss[:], ones_c[:], sq[:],
                                     start=(j == 0), stop=False)
                qacc = [ps.tile([P, S], f32, tag="acc", name=f"qacc_{l}_{m}")
                        for m in range(NMQ)]
                qw = []
                for m in range(NMQ):
                    wt = wqk_p.tile([P, NK, P], f32r, tag="wqk",
                                    name=f"wqk_{l}_{m}")
                    nc.sync.dma_start(
                        wt[:], wqk_d.ap()[l, :, m * NK : (m + 1) * NK, :])
                    qw.append(wt)
                    for k in range(NKA):
                        nc.tensor.matmul(qacc[m][:], wt[:, k, :],
                                         x_sb[:, k, :],
                                         start=(k == 0), stop=False)
                # === b-half lands: finish rms sum + q, then k/v ===
                res_adds(prev, NKA, NK, f"x2b_{l}")
                for j in range(NKA, NK):
                    sq = sq_p.tile([P, S], f32r, tag="sq")
                    nc.vector.tensor_mul(sq[:], x_sb[:, j, :], x_sb[:, j, :])
                    nc.tensor.matmul(ss[:], ones_c[:], sq[:],
                                     start=False, stop=(j == NK - 1))
                for m in range(NMQ):
                    for k in range(NKA, NK):
                        nc.tensor.matmul(qacc[m][:], qw[m][:, k, :],
                                         x_sb[:, k, :],
                                         start=False, stop=(k == NK - 1))
                # rms chain (scalar+DVE) overlaps the q/k matmuls above
                srt = sc1_p.tile([1, S], f32, tag="sc1", name=f"srt_{l}")
                nc.scalar.activation(srt[:], ss[:], AF.Sqrt,
                                     bias=eps_t[:], scale=1.0 / D)
                rms = sc1_p.tile([1, S], f32r, tag="sc1", name=f"rms_{l}")
                with nc.allow_low_precision(reason="f32r feed for PE bcast"):
                    nc.vector.reciprocal(rms[:], srt[:])
                bc_ps = ps.tile([P, S], f32, tag="acc", name=f"bcps_{l}")
                nc.tensor.matmul(bc_ps[:], ones_r[:], rms[:],
                                 start=True, stop=True)
                bc_sb = bcs_p.tile([P, S], f32r, tag="bc", name=f"bc_{l}")
                nc.vector.tensor_copy(bc_sb[:], bc_ps[:])

                q_sb = qk_p.tile([P, NMQ, S], f32r, tag="qk", name=f"q_{l}")
                k_sb = qk_p.tile([P, NMQ, S], f32r, tag="qk", name=f"k_{l}")
                for m in range(NMQ):
                    nc.vector.tensor_mul(q_sb[:, m, :], qacc[m][:], bc_sb[:])
                for m in range(NMQ):
                    wt = wqk_p.tile([P, NK, P], f32r, tag="wqk",
                                    name=f"wqk_{l}_k{m}")
                    nc.sync.dma_start(
                        wt[:],
                        wqk_d.ap()[l, :, (NMQ + m) * NK : (NMQ + m + 1) * NK, :])
                    acc = ps.tile([P, S], f32, tag="acc")
                    for k in range(NK):
                        nc.tensor.matmul(acc[:], wt[:, k, :], x_sb[:, k, :],
                                         start=(k == 0), stop=(k == NK - 1))
                    nc.vector.tensor_mul(k_sb[:, m, :], acc[:], bc_sb[:])

                # --- v (token-major; k-outer, h=x*bc per k, 4 open accs) ---
                v_sb = v_p.tile([P, 4, HL * 65], f32r, tag="v", name=f"v_{l}")
                nc.vector.memset(
                    v_sb.rearrange("p c (h e) -> p c h e", e=65)
                    [:, :, :, 64:65].bitcast(f32), 1.0)
                vacc = [ps.tile([P, QS], f32, tag="acc", name=f"vacc_{l}_{c}")
                        for c in range(4)]
                for k in range(NK):
                    hk = sq_p.tile([P, S], f32r, tag="sq", name=f"hk_{l}_{k}")
                    nc.vector.tensor_mul(hk[:], x_sb[:, k, :], bc_sb[:])
                    for c in range(4):
                        nc.tensor.matmul(vacc[c][:],
                                         hk[:, c * P : (c + 1) * P],
                                         wv[:, k, :],
                                         start=(k == 0), stop=(k == NK - 1))
                for c in range(4):
                    nc.vector.tensor_copy(
                        v_sb.rearrange("p c (h e) -> p c h e", e=65)[:, c, :, :64],
                        vacc[c][:].rearrange("p (h e) -> p h e", e=DK),
                    )

                # --- attention pairs with interleaved wo m0-3 wave ---
                on_sb = on_p.tile([P, NMQ, S], f32r, tag="on", name=f"on_{l}")
                nc.vector.memset(on_sb[64:, NMQ - 1, :].bitcast(f32), 0.0)
                wom = []
                for m in range(NKA):
                    wt = wo_p.tile([P, 3, P], f32r, tag="wo", name=f"wo_{l}_{m}")
                    nc.sync.dma_start(
                        wt[:], wo_d.ap()[l, :, m * 3 : (m + 1) * 3, :])
                    wom.append(wt)
                woacc = [ps.tile([P, S], f32, tag="acc", name=f"woacc_{l}_{m}")
                         for m in range(NKA)]
                part_sb = big_p.tile([P, NK, S], f16, tag="big",
                                     name=f"part_a_{l}")
                def norm_and_wave(pi, pair, avs):
                    # softmax normalization of a finished pair + its wo-wave
                    # matmuls; emitted AFTER the next pair's matmuls so the
                    # strict-FIFO PE never stalls on the DVE recip / osc DMA
                    for hh in pair:
                        base = (hh % 2) * 64
                        rcp = sc1_p.tile([1, S], f32r, tag="rcp", bufs=2,
                                         name=f"rcp_{l}_{hh}")
                        with nc.allow_low_precision(reason="f32r bcast feed"):
                            nc.vector.reciprocal(rcp[:], avs[hh][64:65, :])
                        rb_ps = ps.tile([64, S], f32, tag="sc", bufs=2)
                        nc.tensor.matmul(rb_ps[:], ones_r[:, :64], rcp[:],
                                         start=True, stop=True)
                        rb = rb_p.tile([64, S], f32, tag="rb")
                        nc.vector.tensor_copy(rb[:], rb_ps[:])
                        if base == 0:
                            nc.vector.tensor_mul(on_sb[:64, pi, :],
                                                 avs[hh][:64, :], rb[:])
                        else:
                            osc = onorm_p.tile([64, S], f32r, tag="onorm")
                            nc.vector.tensor_mul(osc[:], avs[hh][:64, :], rb[:])
                            nc.sync.dma_start(on_sb[64:, pi, :], osc[:])
                    for m in range(NKA):
                        nc.tensor.matmul(woacc[m][:], wom[m][:, pi, :],
                                         on_sb[:, pi, :],
                                         start=(pi == 0), stop=(pi == 2))

                for pi, pair in enumerate(((0, 1), (2, 3), (4,))):
                    av = {}
                    for hh in pair:
                        av[hh] = ps.tile([65, S], f32, tag="av", bufs=2,
                                         name=f"av_{l}_{hh}")
                    for cch in range(4):
                        for hh in pair:
                            base = (hh % 2) * 64
                            s_ps = ps.tile([P, S], f32, tag="sc", bufs=2)
                            nc.tensor.matmul(
                                s_ps[:],
                                k_sb[base : base + DK, pi, cch * P : (cch + 1) * P],
                                q_sb[base : base + DK, pi, :],
                                start=True, stop=True,
                            )
                            if l == 0:
                                pbt = pb_p.tile([P, S], bf16, tag="pb")
                                nc.sync.dma_start(pbt[:], posb_d.ap()[hh, cch])
                                nc.vector.tensor_add(s_ps[:], s_ps[:], pbt[:])
                            e_t = e_p.tile([P, S], f32r, tag="e")
                            nc.scalar.activation(e_t[:], s_ps[:], AF.Exp,
                                                 bias=ebias_t[:], scale=1.0)
                            nc.tensor.matmul(
                                av[hh][:],
                                v_sb[:, cch, hh * 65 : hh * 65 + 65],
                                e_t[:],
                                start=(cch == 0), stop=(cch == 3),
                            )
                    norm_and_wave(pi, pair, av)
                for m in range(NKA):
                    nc.vector.tensor_copy(part_sb[:, m, :], woacc[m][:])
                co1a = fire_ar(part_sb, 0, NKA, f"ar1a_{l}")
                for m in range(NKA, NK):
                    wt = wo_p.tile([P, 3, P], f32r, tag="wo", name=f"wo_{l}_{m}")
                    nc.sync.dma_start(
                        wt[:], wo_d.ap()[l, :, m * 3 : (m + 1) * 3, :])
                    acc = ps.tile([P, S], f32, tag="acc")
                    for k3 in range(3):
                        nc.tensor.matmul(acc[:], wt[:, k3, :], on_sb[:, k3, :],
                                         start=(k3 == 0), stop=(k3 == 2))
                    nc.vector.tensor_copy(part_sb[:, m, :], acc[:])
                co1b = fire_ar(part_sb, NKA, NK, f"ar1b_{l}")
                warm_chain(12, f"a1_{l}")

                # === FFN; w1 k-split waves consume AR1 halves ===
                res_adds((co1a, co1b), 0, NKA, f"x1a_{l}")
                wpairs = [(0, 5), (1, 6), (2, 7), (3, 8), (4, 9)]
                w1w, w1acc = {}, {}
                for i, (ma, mg) in enumerate(wpairs[:2]):
                    for mm in (ma, mg):
                        wt = w1_p.tile([P, NK, P], f32r, tag="w1",
                                       name=f"w1_{l}_{mm}")
                        nc.sync.dma_start(
                            wt[:], w1_d.ap()[l, :, mm * NK : (mm + 1) * NK, :])
                        w1w[mm] = wt
                        w1acc[mm] = ps.tile([P, S], f32, tag="acc",
                                            name=f"w1acc_{l}_{mm}")
                        for k in range(NKA):
                            nc.tensor.matmul(w1acc[mm][:], wt[:, k, :],
                                             x_sb[:, k, :],
                                             start=(k == 0), stop=False)
                res_adds((co1a, co1b), NKA, NK, f"x1b_{l}")
                ff_sb = ff_p.tile([P, 5, S], f32r, tag="ff", name=f"ff_{l}")
                for i, (ma, mg) in enumerate(wpairs):
                    if i < 2:
                        for mm in (ma, mg):
                            for k in range(NKA, NK):
                                nc.tensor.matmul(w1acc[mm][:], w1w[mm][:, k, :],
                                                 x_sb[:, k, :],
                                                 start=False, stop=(k == NK - 1))
                        a_ps, g_ps = w1acc[ma], w1acc[mg]
                    else:
                        accs = []
                        for mm in (ma, mg):
                            wt = w1_p.tile([P, NK, P], f32r, tag="w1",
                                           name=f"w1_{l}_{mm}")
                            nc.sync.dma_start(
                                wt[:],
                                w1_d.ap()[l, :, mm * NK : (mm + 1) * NK, :])
                            acc = ps.tile([P, S], f32, tag="acc")
                            for k in range(NK):
                                nc.tensor.matmul(acc[:], wt[:, k, :],
                                                 x_sb[:, k, :],
                                                 start=(k == 0),
                                                 stop=(k == NK - 1))
                            accs.append(acc)
                        a_ps, g_ps = accs
                    ga = ga_p.tile([P, S], f32r, tag="ga")
                    nc.scalar.activation(ga[:], g_ps[:], AF.Gelu_apprx_tanh)
                    nc.vector.tensor_mul(ff_sb[:, i, :], a_ps[:], ga[:])

                # === w2 with m0-3 wave feeding AR2a early ===
                part2_sb = big_p.tile([P, NK, S], f16, tag="big",
                                      name=f"part_f_{l}")
                w2m = []
                for m in range(NKA):
                    wt = w2_p.tile([P, 5, P], f32r, tag="w2",
                                   name=f"w2_{l}_{m}")
                    nc.sync.dma_start(
                        wt[:], w2_d.ap()[l, :, m * 5 : (m + 1) * 5, :])
                    w2m.append(wt)
                w2acc = [ps.tile([P, S], f32, tag="acc",
                                 name=f"w2acc_{l}_{m}") for m in range(NKA)]
                for k5 in range(5):
                    for m in range(NKA):
                        nc.tensor.matmul(w2acc[m][:], w2m[m][:, k5, :],
                                         ff_sb[:, k5, :],
                                         start=(k5 == 0), stop=(k5 == 4))
                for m in range(NKA):
                    nc.vector.tensor_copy(part2_sb[:, m, :], w2acc[m][:])
                co2a = fire_ar(part2_sb, 0, NKA, f"ar2a_{l}")
                for m in range(NKA, NK):
                    wt = w2_p.tile([P, 5, P], f32r, tag="w2",
                                   name=f"w2_{l}_{m}")
                    nc.sync.dma_start(
                        wt[:], w2_d.ap()[l, :, m * 5 : (m + 1) * 5, :])
                    acc = ps.tile([P, S], f32, tag="acc")
                    for k5 in range(5):
                        nc.tensor.matmul(acc[:], wt[:, k5, :], ff_sb[:, k5, :],
                                         start=(k5 == 0), stop=(k5 == 4))
                    nc.vector.tensor_copy(part2_sb[:, m, :], acc[:])
                co2b = fire_ar(part2_sb, NKA, NK, f"ar2b_{l}")
                prev = (co2a, co2b)

            # ---------------- final norm + classifier ----------------
            # epilogue of the last AR2 with a classifier k-split wave;
            # the final 1/rms commutes through the classifier: scale logits
            res_adds(prev, 0, NKA, "fin_a")
            ss = ps.tile([1, S], f32, tag="acc", name="ss_fin")
            for j in range(NKA):
                sq = sq_p.tile([P, S], f32r, tag="sq")
                nc.vector.tensor_mul(sq[:], x_sb[:, j, :], x_sb[:, j, :])
                nc.tensor.matmul(ss[:], ones_c[:], sq[:],
                                 start=(j == 0), stop=False)
            NCW = 2
            clacc = [ps.tile([P, S], f32, tag="acc", name=f"clacc_{m}")
                     for m in range(NCW)]
            clw = []
            for m in range(NCW):
                cw = clsw_p.tile([P, NK, P], f32r, tag="clsw",
                                 name=f"clsw_{m}")
                nc.sync.dma_start(cw[:],
                                  cls_d.ap()[:, m * NK : (m + 1) * NK, :])
                clw.append(cw)
                for k in range(NKA):
                    nc.tensor.matmul(clacc[m][:], cw[:, k, :], x_sb[:, k, :],
                                     start=(k == 0), stop=False)
            res_adds(prev, NKA, NK, "fin_b")
            for j in range(NKA, NK):
                sq = sq_p.tile([P, S], f32r, tag="sq")
                nc.vector.tensor_mul(sq[:], x_sb[:, j, :], x_sb[:, j, :])
                nc.tensor.matmul(ss[:], ones_c[:], sq[:],
                                 start=False, stop=(j == NK - 1))
            srt = sc1_p.tile([1, S], f32, tag="sc1", name="srt_fin")
            nc.scalar.activation(srt[:], ss[:], AF.Sqrt,
                                 bias=eps_t[:], scale=1.0 / D)
            rms = sc1_p.tile([1, S], f32r, tag="sc1", name="rms_fin")
            with nc.allow_low_precision(reason="f32r feed for PE bcast"):
                nc.vector.reciprocal(rms[:], srt[:])
            bc_ps = ps.tile([P, S], f32, tag="acc", name="bcps_fin")
            nc.tensor.matmul(bc_ps[:], ones_r[:], rms[:],
                             start=True, stop=True)
            bc_sb = bcs_p.tile([P, S], f32r, tag="bc", name="bc_fin")
            nc.vector.tensor_copy(bc_sb[:], bc_ps[:])

            for m in range(NMC):
                if m < NCW:
                    cw = clw[m]
                    acc = clacc[m]
                    for k in range(NKA, NK):
                        nc.tensor.matmul(acc[:], cw[:, k, :], x_sb[:, k, :],
                                         start=False, stop=(k == NK - 1))
                else:
                    cw = clsw_p.tile([P, NK, P], f32r, tag="clsw")
                    nc.sync.dma_start(
                        cw[:], cls_d.ap()[:, m * NK : (m + 1) * NK, :])
                    acc = ps.tile([P, S], f32, tag="acc")
                    for k in range(NK):
                        nc.tensor.matmul(acc[:], cw[:, k, :], x_sb[:, k, :],
                                         start=(k == 0), stop=(k == NK - 1))
                ot = outw_p.tile([P, S], f32, tag="outw")
                nc.vector.tensor_mul(ot[:], acc[:], bc_sb[:])
                nc.vector.tensor_scalar_add(ot[:], ot[:], clsb_sb[:, m : m + 1])
                nc.sync.dma_start(out_d.ap()[m], ot[:])

    nc.compile()
    _BUILD_CACHE["nc"] = nc
    return nc


def _run(in_maps, **kw):
    from concourse import bass_utils

    nc = build_nc()
    return bass_utils.run_bass_kernel_spmd(
        nc, in_maps, core_ids=list(range(N_CORES)), **kw
    )


def kernel(**inputs):
    in_maps = make_in_maps(inputs)
    res = _run(in_maps)
    return assemble_output(res.results)


def assemble_output(results):
    full = np.empty((B, VOCAB * NPRED, S), np.float32)
    for c in range(N_CORES):
        g, r = divmod(c, TP)
        full[g, r * CS : (r + 1) * CS] = (
            np.asarray(results[c]["out"], np.float32).reshape(CS, S)
        )
    out = (
        full.reshape(B, VOCAB, NPRED, S)
        .transpose(0, 1, 3, 2)
        .reshape(B, VOCAB, S * NPRED)
    )
    return np.ascontiguousarray(out)
